# revision 27
# baseline (speedup 1.0000x reference)
"""AudioAttentionPooler Trainium2 kernel.

Algorithm (algebraically identical to the reference, ~60x fewer FLOPs):
  scores[b,t,h] = x[b,t,:] @ Wq[:,h]        Wq = fold(query*scale, kv_w_k)  [C,h]
  (k-bias shifts scores uniformly along t -> softmax-invariant -> dropped)
  e = exp(scores)                           (mask folded into x and Z instead)
  Z[b,h] = sum_t e[b,t,h] * mask[b,t]
  px[b,h,:] = sum_t e[b,t,h] * (mask[b,t] * x[b,t,:])   (pool BEFORE v-proj)
  out1[b,h*64+d] = (px[b,h,:] @ Wv[:,h*64+d]) / Z[b,h]
  out = out1 @ out_w + (kv_b_v @ out_w + out_b)   (v-bias exact: attn sums to 1)

Sharding: data-parallel over batch, 4 batch elements per core x 8 cores.
x is fed in both [T,C] and [C,T] layouts (host transpose) because the PE
contracts over the partition dim: scores contract over C, pooling over T.
"""

import numpy as np
import ml_dtypes

BF16 = ml_dtypes.bfloat16

HIDDEN = 1024
NH = 16
HD = 64
PROJ = 1024
B, T = 32, 2048
NCORES = 8
NB = B // NCORES          # 4 batch elems per core
KT = HIDDEN // 128        # 8 C-tiles
MT = T // 128             # 16 T-chunks
F8 = ml_dtypes.float8_e4m3
F8MAX = 240.0             # conservative e4m3 range cap

_CACHED_NC = None


def _build_nc(reps=1):
    import concourse.bacc as bacc
    import concourse.mybir as mybir
    import concourse.tile as tile

    f32 = mybir.dt.float32
    bf16 = mybir.dt.bfloat16
    f8 = mybir.dt.float8e4

    nc = bacc.Bacc("TRN2", target_bir_lowering=False, debug=False)

    x_d = nc.dram_tensor("x", [NB, T, HIDDEN], bf16, kind="ExternalInput")
    xt_d = nc.dram_tensor("xt", [NB, 128, MT, KT, 128], f8, kind="ExternalInput")
    wq_d = nc.dram_tensor("wq", [128, KT, NH], f8, kind="ExternalInput")
    wv_d = nc.dram_tensor("wv", [128, KT, NH, HD], bf16, kind="ExternalInput")
    wo_d = nc.dram_tensor("wo", [128, KT, 2, 512], bf16, kind="ExternalInput")
    mcol_d = nc.dram_tensor("mcol", [128, NB, MT], bf16, kind="ExternalInput")
    biasrep_d = nc.dram_tensor("biasrep", [NB, PROJ], f32, kind="ExternalInput")
    onescol_d = nc.dram_tensor("onescol", [1, 128], f32, kind="ExternalInput")
    idf_d = nc.dram_tensor("idf", [128, 128], f32, kind="ExternalInput")
    escale_d = nc.dram_tensor("escale", [128, 1], f32, kind="ExternalInput")
    out_d = nc.dram_tensor("out", [NB, PROJ], f32, kind="ExternalOutput")

    from contextlib import nullcontext

    with tile.TileContext(nc) as tc:
        with (
            tc.tile_pool(name="consts", bufs=1) as consts,
            tc.tile_pool(name="xpool", bufs=3) as xpool,
            tc.tile_pool(name="xtpool", bufs=3) as xtpool,
            tc.tile_pool(name="work", bufs=3) as work,
            tc.tile_pool(name="small", bufs=1) as small,
            tc.tile_pool(name="scps", bufs=2, space="PSUM") as scps,
            tc.tile_pool(name="pxps", bufs=2, space="PSUM") as pxps,
            tc.tile_pool(name="tps", bufs=2, space="PSUM") as tps,
            tc.tile_pool(name="bigps", bufs=1, space="PSUM") as bigps,
        ):
            wq_sb = consts.tile([128, KT, NH], f8)
            wv_sb = consts.tile([128, KT, NH, HD], bf16)
            wo_sb = consts.tile([128, KT, 2, 512], bf16)
            mcol_sb = consts.tile([128, NB, MT], bf16)
            biasrep_sb = consts.tile([NB, PROJ], f32)
            onescol_sb = consts.tile([1, 128], f32)
            idf_sb = consts.tile([128, 128], f32)
            escale_sb = consts.tile([128, 1], f32)
            nc.sync.dma_start(wq_sb[:], wq_d[:])

            # persistent accumulators across the b-loop
            pxall_sb = small.tile([128, KT, NH, NB], bf16)

            rep_ctx = tc.For_i(0, reps, 1) if reps > 1 else nullcontext()
            with rep_ctx:
              for b in range(NB):
                  x_sb = xpool.tile([128, MT, HIDDEN], bf16)
                  xt_sb = xtpool.tile([128, MT, KT, 128], f8)
                  for m4 in range(4):
                      nc.sync.dma_start(
                          xt_sb[:, m4 * 4:(m4 + 1) * 4],
                          xt_d[b, :, m4 * 4:(m4 + 1) * 4],
                      )
                  for m4 in range(4):
                      nc.sync.dma_start(
                          x_sb[:, m4 * 4:(m4 + 1) * 4],
                          x_d[b, m4 * 512:(m4 + 1) * 512].rearrange(
                              "(m p) c -> p m c", p=128
                          ),
                      )
                  # deferred const loads, ordered by first use so early DMA
                  # bandwidth goes to the batch data stream; stage-3/4 weights
                  # stream per-k AFTER all batch data so the pooling loop is
                  # never delayed and stage 3/4 chase the weight chunks
                  if b == 0:
                      nc.sync.dma_start(escale_sb[:], escale_d[:])
                      nc.sync.dma_start(onescol_sb[:], onescol_d[:])
                      nc.sync.dma_start(mcol_sb[:], mcol_d[:])
                      nc.sync.dma_start(idf_sb[:], idf_d[:])
                  elif b == NB - 1:
                      for k in range(KT):
                          nc.sync.dma_start(wv_sb[:, k], wv_d[:, k])
                      for k in range(KT):
                          nc.sync.dma_start(wo_sb[:, k], wo_d[:, k])
                      nc.sync.dma_start(biasrep_sb[:], biasrep_d[:])

                  # --- scores[t, h] = x @ Wq ---------------------------------
                  sc_sb = work.tile([128, MT, NH], f32)
                  for m2 in range(MT // 4):
                      sc_ps = scps.tile([128, 4, NH], f32, tag="sc")
                      for m4 in range(4):
                          m = m2 * 4 + m4
                          for k in range(KT):
                              nc.tensor.matmul(
                                  sc_ps[:, m4, :],
                                  xt_sb[:, m, k, :],
                                  wq_sb[:, k, :],
                                  start=(k == 0),
                                  stop=(k == KT - 1),
                              )
                      nc.vector.tensor_copy(sc_sb[:, m2 * 4:(m2 + 1) * 4, :], sc_ps[:])

                  # --- e = exp(scores) (bf16); mask is folded into x and the
                  # Z moving operand, so no explicit mask multiply is needed.
                  # Split into per-group ops so the pooling matmuls can trail
                  # the score stream instead of waiting for all 16 chunks -----
                  e_sb = work.tile([128, MT, NH], bf16)
                  for m2 in range(MT // 4):
                      nc.scalar.activation(
                          e_sb[:, m2 * 4:(m2 + 1) * 4, :],
                          sc_sb[:, m2 * 4:(m2 + 1) * 4, :],
                          mybir.ActivationFunctionType.Exp,
                          scale=escale_sb[:],
                      )

                  # --- Z[h] = sum_t e (output oriented [NH, 1]) --------------
                  z_ps = tps.tile([NH, 1], f32, tag="tps")
                  for m in range(MT):
                      nc.tensor.matmul(
                          z_ps[:],
                          e_sb[:, m, :],
                          mcol_sb[:, b, m:m + 1],
                          start=(m == 0),
                          stop=(m == MT - 1),
                      )
                  z_sb = work.tile([NH, 1], f32)
                  nc.vector.tensor_copy(z_sb[:], z_ps[:])
                  # broadcast 1/Z down all 128 partitions: [NH,1] -T-> [1,NH]
                  # -K=1 matmul-> [128,NH] -reciprocal-> sbuf
                  zt_ps = tps.tile([1, NH], f32, tag="tps")
                  nc.tensor.transpose(zt_ps[:], z_sb[:], idf_sb[0:NH, 0:NH])
                  zt_sb = work.tile([1, NH], f32)
                  nc.vector.tensor_copy(zt_sb[:], zt_ps[:])
                  zbc_ps = tps.tile([128, NH], f32, tag="tps")
                  nc.tensor.matmul(
                      zbc_ps[:], onescol_sb[:], zt_sb[:], start=True, stop=True
                  )
                  zinv_sb = work.tile([128, NH], f32)
                  nc.vector.reciprocal(zinv_sb[:], zbc_ps[:])

                  # --- px[h, c] = e.T @ x (unnormalized pool) ----------------
                  px_sb = work.tile([NH, HIDDEN], f32)
                  for c2 in range(2):
                      px_ps = pxps.tile([NH, 512], f32, tag="px")
                      for m in range(MT):
                          nc.tensor.matmul(
                              px_ps[:],
                              e_sb[:, m, :],
                              x_sb[:, m, c2 * 512:(c2 + 1) * 512],
                              start=(m == 0),
                              stop=(m == MT - 1),
                          )
                      nc.vector.tensor_copy(px_sb[:, c2 * 512:(c2 + 1) * 512], px_ps[:])

                  # --- pxT: [C-tile, h] with b packed in the free dim --------
                  for k in range(KT):
                      pxt_ps = tps.tile([128, NH], f32, tag="tps")
                      nc.tensor.transpose(
                          pxt_ps[:], px_sb[:, k * 128:(k + 1) * 128], idf_sb[0:NH, 0:NH]
                      )
                      nc.vector.tensor_mul(pxall_sb[:, k, :, b], pxt_ps[:], zinv_sb[:])

              # --- stage 3: out1_raw[b, hd] = px @ Wv -------------------------
              out1_ps = bigps.tile([NB, HIDDEN], f32)
              for h in range(NH):
                  for k in range(KT):
                      nc.tensor.matmul(
                          out1_ps[:, h * HD:(h + 1) * HD],
                          pxall_sb[:, k, h, :],
                          wv_sb[:, k, h, :],
                          start=(k == 0),
                          stop=(k == KT - 1),
                      )

              # --- out1T: [hd-tile, b] (out1 already normalized; per-k copies
              # so the copy/transpose/stage-4 chain trails stage 3 head-by-head
              # instead of waiting for the full [4,1024] psum) -----------------
              out1n_sb = small.tile([NB, HIDDEN], f32)
              o1t_sb = small.tile([128, KT, NB], bf16)
              for k in range(KT):
                  nc.vector.tensor_copy(
                      out1n_sb[:, k * 128:(k + 1) * 128],
                      out1_ps[:, k * 128:(k + 1) * 128],
                  )
                  o1t_ps = tps.tile([128, NB], f32, tag="tps")
                  nc.tensor.transpose(
                      o1t_ps[:], out1n_sb[:, k * 128:(k + 1) * 128], idf_sb[0:NB, 0:NB]
                  )
                  nc.vector.tensor_copy(o1t_sb[:, k, :], o1t_ps[:])

              # --- stage 4: out = out1 @ out_w + bias -------------------------
              of_sb = small.tile([NB, PROJ], f32)
              of_ps0 = scps.tile([NB, 512], f32, tag="sc")
              of_ps1 = scps.tile([NB, 512], f32, tag="sc")
              for k in range(KT):
                  for p2, of_ps in ((0, of_ps0), (1, of_ps1)):
                      nc.tensor.matmul(
                          of_ps[:],
                          o1t_sb[:, k, :],
                          wo_sb[:, k, p2, :],
                          start=(k == 0),
                          stop=(k == KT - 1),
                      )
              for p2, of_ps in ((0, of_ps0), (1, of_ps1)):
                  nc.vector.tensor_add(
                      of_sb[:, p2 * 512:(p2 + 1) * 512],
                      of_ps[:],
                      biasrep_sb[:, p2 * 512:(p2 + 1) * 512],
                  )
              nc.sync.dma_start(out_d[:], of_sb[:])

    nc.compile()
    return nc


def _get_nc():
    global _CACHED_NC
    if _CACHED_NC is None:
        _CACHED_NC = _build_nc()
    return _CACHED_NC


def _prep_inputs(hidden_states, mask, kv_w, kv_b, out_w, out_b, query):
    """Host-side sharding + weight preprocessing -> per-core input maps."""
    x = np.ascontiguousarray(hidden_states, dtype=np.float32)
    mask = np.asarray(mask)
    kv_w = np.asarray(kv_w, dtype=np.float32)
    kv_b = np.asarray(kv_b, dtype=np.float32)
    out_w = np.asarray(out_w, dtype=np.float32)
    out_b = np.asarray(out_b, dtype=np.float32)
    query = np.asarray(query, dtype=np.float32)

    scale = 1.0 / HD ** 0.5
    Wk = kv_w[:, :HIDDEN]
    Wv = kv_w[:, HIDDEN:]
    qh = query.reshape(NH, HD)
    # fold query into the k-projection: Wq[c, h]
    Wq = np.einsum("chd,hd->ch", Wk.reshape(HIDDEN, NH, HD), qh) * scale
    bias_final = kv_b[HIDDEN:] @ out_w + out_b  # v-bias is exact post-pool

    # dynamic power-of-2 fp8 scales (exactly unwound inside the exp activation)
    sw = 2.0 ** np.floor(np.log2(F8MAX / max(np.abs(Wq).max(), 1e-30)))
    sx = 2.0 ** np.floor(np.log2(F8MAX / max(np.abs(x).max(), 1e-30)))
    sx = min(sx, 1.0)
    escale = np.full((128, 1), 1.0 / (sw * sx), np.float32)
    wq_r = np.ascontiguousarray(
        (Wq * sw).reshape(KT, 128, NH).transpose(1, 0, 2)
    ).astype(F8)  # [128, KT, NH], fp8 with exp-unwound scale
    wv_r = np.ascontiguousarray(
        Wv.reshape(KT, 128, NH, HD).transpose(1, 0, 2, 3)
    ).astype(BF16)  # [128, KT, NH, HD]
    wo_r = np.ascontiguousarray(
        out_w.reshape(KT, 128, 2, 512).transpose(1, 0, 2, 3)
    ).astype(BF16)  # [128, KT, 2, 512]
    onescol = np.ones((1, 128), np.float32)
    idf = np.eye(128, dtype=np.float32)

    mvalid = (mask != 0).astype(np.float32)      # reference masks where mask == 0
    x_bf = (x * mvalid[:, :, None]).astype(BF16)  # pre-masked pooling copy [B, T, C]
    # xt chunked layout: xtr[b, p, m, k, t] = x[b, m*128+t, k*128+p]; per-partition
    # rows are contiguous in (m, k, t) so m-range DMA slices stay order-aligned
    xt_bf = np.ascontiguousarray(
        (x * sx).reshape(B, MT, 128, KT, 128).transpose(0, 4, 1, 3, 2)
    ).astype(F8)

    in_maps = []
    for c in range(NCORES):
        sl = slice(c * NB, (c + 1) * NB)
        # mcol[p, b, m] = valid(mask[b, m*128+p])
        mcol = np.ascontiguousarray(
            mvalid[sl].reshape(NB, MT, 128).transpose(2, 0, 1)
        ).astype(BF16)
        in_maps.append({
            "x": x_bf[sl],
            "xt": xt_bf[sl],
            "wq": wq_r,
            "wv": wv_r,
            "wo": wo_r,
            "mcol": mcol,
            "biasrep": np.ascontiguousarray(
                np.broadcast_to(bias_final[None, :], (NB, PROJ))
            ),
            "onescol": onescol,
            "idf": idf,
            "escale": escale,
        })
    return in_maps


def kernel(hidden_states, mask, kv_w, kv_b, out_w, out_b, query, **_unused):
    from concourse.bass_utils import run_bass_kernel_spmd

    nc = _get_nc()
    in_maps = _prep_inputs(hidden_states, mask, kv_w, kv_b, out_w, out_b, query)
    res = run_bass_kernel_spmd(nc, in_maps, list(range(NCORES)))
    out = np.concatenate([res.results[i]["out"] for i in range(NCORES)], axis=0)
    return out.astype(np.float32)

